# revision 1
# baseline (speedup 1.0000x reference)
"""Jacobi 100-step solver on 8 trn2 cores via truncated DST-spectral transform.

x_{t+1} = mask * (0.25 * 4-neighbor-sum) is linear and diagonalizes in the DST
basis Q: after one explicit step, x100 = Q (s^99 . (Q x1 Q)) Q with
s = 0.5(cos a + cos b). |s|^99 is negligible outside the lowest-K and highest-K
mode corners (K=256 -> rel err ~6.5e-3 incl fp16 noise, gate 2e-2). Everything
on-device runs in fp16 (1-pass PE matmuls, fp32 PSUM accumulation); the
spectral AllReduce payload is one [512,256] fp16 block. Sharding: 256-column
panels per core. DMA triggers cost ~600ns each serialized on the sync queue,
so loads are batched into few large rearranged transfers.
"""

import sys
import types
import numpy as np

N = 2048
NC = 8
P = N // NC          # 256 panel columns per core
K = 256              # spectral corner size per corner
K2 = 2 * K           # lo|hi concatenated
PW = P + 2           # panel width with 1-col halos
PW2 = 2 * PW         # X|Y interleaved row width
RC = N // 128        # 16 row chunks


def _install_ntff_hook():
    if "antenv.axon_hooks" in sys.modules:
        return
    mod = types.ModuleType("antenv.axon_hooks")
    mod._hook = None
    mod.set_axon_ntff_profile_hook = lambda h: setattr(mod, "_hook", h)
    mod.get_axon_ntff_profile_hook = lambda: mod._hook
    sys.modules["antenv.axon_hooks"] = mod
    try:
        import antenv
        antenv.axon_hooks = mod
        from trn_agent_boot.trn_boot import _ntff_profile_via_ctypes
        h = _ntff_profile_via_ctypes("/opt/axon/libaxon_pjrt.so")
        if h is not None:
            mod.set_axon_ntff_profile_hook(h)
    except Exception:
        pass


def _host_constants():
    # hi modes in DESCENDING order (m = 2046..1791) so that
    # Qc_hi = diag((-1)^(i+1)) @ Qc_lo (DST checkerboard identity):
    # only the lo basis is uploaded; hi is derived on-device by sign flips.
    i = np.arange(N, dtype=np.float64)
    qcs, qcTs, w99s = [], [], []
    for lo in (True, False):
        m = np.arange(1, K + 1, dtype=np.float64) if lo else np.arange(N - 2, N - 2 - K, -1, dtype=np.float64)
        red = np.outer(i, m) % (2 * (N - 1))
        Qc = np.sqrt(2.0 / (N - 1)) * np.sin(np.pi * red / (N - 1))   # [2048, K]
        lam = 0.5 * np.cos(np.pi * m / (N - 1))
        W99 = (lam[:, None] + lam[None, :]) ** 99                     # [K, K]
        qcs.append(Qc.astype(np.float16))
        qcTs.append(np.ascontiguousarray(Qc.T).astype(np.float16))
        w99s.append(W99.astype(np.float16))
    sident = np.zeros((128, 128), np.float16)
    for p in range(128):
        sident[p, p] = -1.0 if p % 2 == 0 else 1.0   # (-1)^(p+1)
    consts = {
        "qcb": qcs[0],                                                # [2048, 256] lo only
        "qcTb": np.ascontiguousarray(np.concatenate(qcTs, axis=0)),   # [512, 2048]
        "w99b": np.ascontiguousarray(np.concatenate(w99s, axis=0)),   # [512, 256]
        "ident": np.eye(128, dtype=np.float16),
        "sident": sident,
    }
    smid = np.zeros((128, 128), np.float16)
    for d in range(127):
        smid[d, d + 1] = 1.0
        smid[d + 1, d] = 1.0
    sup = np.zeros((128, 128), np.float16); sup[127, 0] = 1.0
    sdn = np.zeros((128, 128), np.float16); sdn[0, 127] = 1.0
    sgncol = np.array([[-1.0 if p % 2 == 0 else 1.0] for p in range(128)], np.float16)
    consts["sss"] = np.ascontiguousarray(np.concatenate([smid, sup, sdn, sgncol], axis=1))  # [128, 385]
    return consts


_NC_CACHE = {}


def _build():
    if "nc" in _NC_CACHE:
        return _NC_CACHE["nc"]
    import concourse.bacc as bacc
    import concourse.tile as tile
    import concourse.mybir as mybir

    F16 = mybir.dt.float16
    F32 = mybir.dt.float32
    nc = bacc.Bacc("TRN2", target_bir_lowering=False, debug=False, num_devices=NC)

    sss_d = nc.dram_tensor("sss", [128, 385], F16, kind="ExternalInput")
    xy_d = nc.dram_tensor("xy", [N, PW2], F16, kind="ExternalInput")
    qcb_d = nc.dram_tensor("qcb", [N, K], F16, kind="ExternalInput")
    qrowsb_d = nc.dram_tensor("qrowsb", [P, K], F16, kind="ExternalInput")
    w99b_d = nc.dram_tensor("w99b", [K2, K], F16, kind="ExternalInput")
    qrowsTb_d = nc.dram_tensor("qrowsTb", [K2, P], F16, kind="ExternalInput")
    ident_d = nc.dram_tensor("ident", [128, 128], F16, kind="ExternalInput")
    sident_d = nc.dram_tensor("sident", [128, 128], F16, kind="ExternalInput")
    out_d = nc.dram_tensor("out", [N, P], F16, kind="ExternalOutput")

    ACTF = mybir.ActivationFunctionType
    LN025 = float(np.log(0.25))

    with tile.TileContext(nc) as tc:
        with tc.tile_pool(name="pers", bufs=1) as pers, \
             tc.tile_pool(name="rot", bufs=6) as rot, \
             tc.tile_pool(name="ps", bufs=1, space="PSUM") as ps, \
             tc.tile_pool(name="dram", bufs=2, space="DRAM") as dram:

            # ---- persistent SBUF ----
            x0b = pers.tile([128, RC * PW], F16, tag="x0b")
            x1b = pers.tile([128, RC * P], F16, tag="x1b")
            x1db = pers.tile([128, RC * P], F16, tag="x1db")
            qcb_s = pers.tile([128, RC * K], F16, tag="qcb")
            qcTb_s = pers.tile([128, 4 * N], F16, tag="qcTb")
            qrowsb_s = pers.tile([128, 2 * K], F16, tag="qrb")
            qrowsTb_s = pers.tile([128, 4 * P], F16, tag="qrtb")
            w99_s = pers.tile([128, 4 * K], F16, tag="w99")
            abuf = pers.tile([128, 2 * K2], F16, tag="abuf")
            gsb = pers.tile([128, 4 * K], F16, tag="gsb")
            utb = pers.tile([128, 4 * K], F16, tag="utb")
            zbuf = pers.tile([128, 4 * P], F16, tag="zbuf")
            ident_s = pers.tile([128, 128], F16, tag="ident")
            halfbc = pers.tile([128, 1], F16, tag="halfbc")
            nc.gpsimd.memset(halfbc[:], 0.5)
            sident_s = pers.tile([128, 128], F16, tag="sident")
            sgn_ap = pers.tile([128, 1], F32, tag="sgnap")
            sss_s = pers.tile([128, 385], F16, tag="sss")

            # const APs for activation bias values
            for cv, cn in ((-0.5, "cneg05"), (LN025, "cln025")):
                ct = pers.tile([128, 1], F32, tag=cn, name=cn)
                nc.vector.memset(ct[:], cv)
                nc.const_aps.aps[(F32, float(cv))] = ct[:]

            # ---- start-of-kernel barrier: tiny AllReduce that runs on the idle CC
            # engine during forward; its result gates mm2 (see below) ----
            barrier_in = dram.tile([128, 1], F32, tag="barin")
            barrier_out = dram.tile([128, 1], F32, tag="barout", addr_space="Shared")
            zt = rot.tile([128, 1], F32, tag="zt")
            nc.vector.memset(zt[:], 0.0)
            nc.sync.dma_start(barrier_in[:, :], zt[:])
            nc.gpsimd.collective_compute(
                "AllReduce", mybir.AluOpType.add,
                replica_groups=[list(range(NC))],
                ins=[barrier_in.opt()], outs=[barrier_out.opt()],
            )

            # ---- stencil consts ----
            nc.sync.dma_start(sss_s[:], sss_d[:, :])
            nc.sync.dma_start(ident_s[:], ident_d[:, :])
            nc.sync.dma_start(sident_s[:], sident_d[:, :])
            signbc = sss_s[:, 384:385]
            nc.vector.tensor_copy(sgn_ap[:], signbc)
            smid_s = sss_s[:, 0:128]
            sup_s = sss_s[:, 128:256]
            sdn_s = sss_s[:, 256:384]

            # ---- fused forward: phase0 -> hsum -> phase1 -> mm1, software-pipelined
            # per row chunk so no engine FIFO head-of-line-blocks on a cross-engine
            # dependency. ----
            aps = [ps.tile([128, K], F32, tag="aacc", bufs=4, name=f"aps{jj}") for jj in range(4)]
            d2s = {}
            vpss = {}

            def st_qcb(r):
                nc.sync.dma_start(qcb_s[:, K * r:K * (r + 1)], qcb_d[128 * r:128 * (r + 1), :])

            def st_dma(r):
                # ~65-130KB per transfer: single DMA queues run ~17GB/s, so large
                # batches arrive too late; tiny ones waste ~700ns/trigger on the
                # sync queue.
                xyt = rot.tile([128, PW2], F16, tag="xyt")
                if r < 2:
                    nc.sync.dma_start(xyt[0:64, :], xy_d[128 * r:128 * r + 64, :])
                    nc.sync.dma_start(xyt[64:128, :], xy_d[128 * r + 64:128 * (r + 1), :])
                else:
                    nc.sync.dma_start(xyt[:], xy_d[128 * r:128 * (r + 1), :])
                st_qcb(r)
                return xyt

            def st_sq(r, xyt):
                xt = xyt[:, 0:PW]
                yt = xyt[:, PW:PW2]
                sqx = rot.tile([128, PW], F32, tag="sqx")
                nc.scalar.activation(sqx[:], xt, ACTF.Square, bias=-0.5, scale=1.0)
                ty = rot.tile([128, PW], F16, tag="ty")
                nc.gpsimd.tensor_sub(ty[:], yt, halfbc[:].to_broadcast((128, PW)))
                return sqx, ty

            def st_d2(r, sqx, ty):
                sqy = rot.tile([128, PW], F32, tag="sqy")
                nc.vector.tensor_mul(sqy[:], ty[:], ty[:])
                d2 = rot.tile([128, PW], F32, tag="d2")
                if r % 2 == 0:
                    nc.gpsimd.tensor_add(d2[:], sqx[:], sqy[:])
                else:
                    nc.vector.tensor_add(d2[:], sqx[:], sqy[:])
                return d2

            def st_exp(r, d2):
                # 0.25*exp(-50 d) == exp(-50 d + ln(1/4))
                nc.scalar.activation(x0b[:, PW * r:PW * (r + 1)], d2[:], ACTF.Exp,
                                     bias=LN025, scale=-50.0)

            def st_stencil(r):
                # full 4-neighbor stencil in one PSUM accumulation: vertical via the
                # shift stationaries, horizontal via identity on column-offset slices
                vps = ps.tile([128, P], F32, tag="pp", bufs=4, name="vps")
                nc.tensor.matmul(vps[:], smid_s, x0b[:, PW * r + 1:PW * r + 1 + P],
                                 start=True, stop=False)
                nc.tensor.matmul(vps[:], ident_s[:], x0b[:, PW * r:PW * r + P],
                                 start=False, stop=False)
                if r > 0:
                    nc.tensor.matmul(vps[:], sup_s, x0b[:, PW * (r - 1) + 1:PW * (r - 1) + 1 + P],
                                     start=False, stop=False)
                if r < RC - 1:
                    nc.tensor.matmul(vps[:], sdn_s, x0b[:, PW * (r + 1) + 1:PW * (r + 1) + 1 + P],
                                     start=False, stop=False)
                nc.tensor.matmul(vps[:], ident_s[:], x0b[:, PW * r + 2:PW * r + 2 + P],
                                 start=False, stop=True)
                return vps

            def st_x1(r, vps):
                nc.vector.tensor_copy(x1b[:, P * r:P * (r + 1)], vps[:])

            def st_x1d(r):
                sl = slice(P * r, P * (r + 1))
                if r % 2 == 0:
                    nc.gpsimd.tensor_mul(x1db[:, sl], x1b[:, sl], signbc.to_broadcast((128, P)))
                else:
                    nc.scalar.activation(x1db[:, sl], x1b[:, sl], ACTF.Copy, bias=0.0,
                                         scale=sgn_ap[:])

            def st_mm1(r):
                for jm in range(2):
                    nc.tensor.matmul(aps[2 * jm][:],
                                     x1b[:, P * r + 128 * jm:P * r + 128 * (jm + 1)],
                                     qcb_s[:, K * r:K * (r + 1)],
                                     start=(r == 0), stop=(r == RC - 1))
                    nc.tensor.matmul(aps[2 * jm + 1][:],
                                     x1db[:, P * r + 128 * jm:P * r + 128 * (jm + 1)],
                                     qcb_s[:, K * r:K * (r + 1)],
                                     start=(r == 0), stop=(r == RC - 1))

            xyts = {}
            for r in range(RC + 5):
                if 0 <= r - 2 < RC:
                    st_exp(r - 2, d2s.pop(r - 2))
                if r < RC:
                    xyts[r] = st_dma(r)
                    d2s[r] = st_sq(r, xyts.pop(r))
                if 0 <= r - 1 < RC:
                    d2s[r - 1] = st_d2(r - 1, *d2s[r - 1])
                if 0 <= r - 3 < RC:
                    vpss[r - 3] = st_stencil(r - 3)
                if 0 <= r - 4 < RC:
                    st_x1(r - 4, vpss.pop(r - 4))
                    st_x1d(r - 4)
                if 0 <= r - 5 < RC:
                    st_mm1(r - 5)

            # ---- remaining const loads: one trigger each (streamed during
            # late-forward / AllReduce window) ----
            nc.sync.dma_start(qrowsb_s[:].rearrange("p (c w) -> p c w", w=K2),
                              qrowsb_d[:, :].rearrange("(c p) w -> p c w", p=128))
            nc.sync.dma_start(w99_s[:].rearrange("p (c w) -> p c w", w=K),
                              w99b_d[:, :].rearrange("(c p) w -> p c w", p=128))
            nc.sync.dma_start(qrowsTb_s[:].rearrange("p (c w) -> p c w", w=P),
                              qrowsTb_d[:, :].rearrange("(c p) w -> p c w", p=128))
            nc.sync.dma_start(ident_s[:], ident_d[:, :])
            nc.sync.dma_start(sident_s[:], sident_d[:, :])
            signbc = sss_s[:, 384:385]
            nc.vector.tensor_copy(sgn_ap[:], signbc)

            for jm in range(2):
                nc.vector.tensor_copy(abuf[:, K2 * jm:K2 * jm + K], aps[2 * jm][:])
                nc.vector.tensor_mul(abuf[:, K2 * jm + K:K2 * (jm + 1)], aps[2 * jm + 1][:],
                                     signbc.to_broadcast((128, K)))

            # ---- mm2: G_t = A_t^T @ Qrows_t -> DRAM for AllReduce (fp16 payload) ----
            gin = dram.tile([K2, K], F16, tag="gin")
            gout = dram.tile([K2, K], F16, tag="gout", addr_space="Shared")
            for ti in range(2):
                for am in range(2):
                    gps = ps.tile([128, K], F32, tag="pp", bufs=4, name="gps")
                    for kj in range(2):
                        nc.tensor.matmul(gps[:],
                                         abuf[:, K2 * kj + K * ti + 128 * am:K2 * kj + K * ti + 128 * (am + 1)],
                                         qrowsb_s[:, K * kj:K * (kj + 1)],
                                         start=(kj == 0), stop=(kj == 1))
                    b = 2 * ti + am
                    nc.scalar.copy(gsb[:, K * b:K * (b + 1)], gps[:])
                    # per-chunk gin writes: 4 parallel 65KB transfers instead of one
                    # 262KB single-queue transfer (~15us) on the AllReduce critical path
                    nc.sync.dma_start(gin[128 * b:128 * (b + 1), :], gsb[:, K * b:K * (b + 1)])
            nc.gpsimd.collective_compute(
                "AllReduce", mybir.AluOpType.add,
                replica_groups=[list(range(NC))],
                ins=[gin.opt()], outs=[gout.opt()],
            )

            # ---- build QcT in SBUF from qcb via PE transposes: runs in the
            # AllReduce dead window, replacing a 2MB host upload ----
            for b in range(4):
                rhs_id = ident_s if b < 2 else sident_s
                for r in range(RC):
                    pst = ps.tile([128, P], F32, tag="pp", bufs=4, name="pst")
                    nc.tensor.matmul(pst[:, 0:128],
                                     qcb_s[:, K * r + 128 * (b % 2):K * r + 128 * (b % 2 + 1)],
                                     rhs_id[:], start=True, stop=True)
                    if r % 2 == 0:
                        nc.vector.tensor_copy(qcTb_s[:, N * b + 128 * r:N * b + 128 * (r + 1)], pst[:, 0:128])
                    else:
                        nc.scalar.copy(qcTb_s[:, N * b + 128 * r:N * b + 128 * (r + 1)], pst[:, 0:128])

            # ---- spectral filter: load G^T via XBAR DMA-transpose, then W99 mul.
            # W99 is symmetric, so the same w99 chunks filter the transposed layout.
            for ti in range(2):
                for bm in range(2):
                    b = 2 * ti + bm
                    traw = rot.tile([128, K], F16, tag="traw")
                    eng = nc.sync if bm == 0 else nc.scalar
                    eng.dma_start_transpose(traw[:], gout[K * ti:K * (ti + 1), 128 * bm:128 * (bm + 1)])
                    nc.vector.tensor_mul(utb[:, K * b:K * (b + 1)], traw[:],
                                         w99_s[:, K * b:K * (b + 1)])

            # ---- B1: Z_t = Uhat_t @ QrowsT_t ----
            for ti in range(2):
                for am in range(2):
                    zps = ps.tile([128, P], F32, tag="pp", bufs=4, name="zps")
                    for kb in range(2):
                        nc.tensor.matmul(zps[:],
                                         utb[:, K * (2 * ti + kb) + 128 * am:K * (2 * ti + kb) + 128 * (am + 1)],
                                         qrowsTb_s[:, P * (2 * ti + kb):P * (2 * ti + kb + 1)],
                                         start=(kb == 0), stop=(kb == 1))
                    nc.vector.tensor_copy(zbuf[:, P * (2 * ti + am):P * (2 * ti + am + 1)], zps[:])

            # ---- B2: out_r = sum_{t,ka} QcT_{t,ka,r}^T @ Z_{t,ka}; DMA out in row pairs ----
            for re in range(RC // 2):
                osb2 = rot.tile([128, 2 * P], F16, tag="osb", name="osb2")
                for half in range(2):
                    r = 2 * re + half
                    ops = ps.tile([128, P], F32, tag="pp", bufs=4, name="ops")
                    for ti in range(2):
                        for ka in range(2):
                            b = 2 * ti + ka
                            nc.tensor.matmul(ops[:],
                                             qcTb_s[:, N * b + 128 * r:N * b + 128 * (r + 1)],
                                             zbuf[:, P * b:P * (b + 1)],
                                             start=(b == 0), stop=(b == 3))
                    if half == 0:
                        nc.vector.tensor_copy(osb2[:, 0:P], ops[:])
                    else:
                        nc.scalar.copy(osb2[:, P:2 * P], ops[:])
                nc.scalar.dma_start(out_d[256 * re:256 * re + 128, :], osb2[:, 0:P])
                nc.scalar.dma_start(out_d[256 * re + 128:256 * (re + 1), :], osb2[:, P:2 * P])

    nc.compile()
    _NC_CACHE["nc"] = nc
    return nc


def _run(X, Y, trace=False):
    _install_ntff_hook()
    from concourse.bass_utils import run_bass_kernel_spmd

    consts = _host_constants()
    Xp = np.zeros((N, N + 2), np.float16); Xp[:, 1:-1] = np.asarray(X, np.float32).astype(np.float16)
    Yp = np.zeros((N, N + 2), np.float16); Yp[:, 1:-1] = np.asarray(Y, np.float32).astype(np.float16)

    in_maps = []
    for c in range(NC):
        xy = np.concatenate([Xp[:, P * c:P * c + PW], Yp[:, P * c:P * c + PW]], axis=1)
        m = {"xy": np.ascontiguousarray(xy),
             "qcb": consts["qcb"],
             "w99b": consts["w99b"],
             "qrowsb": np.ascontiguousarray(consts["qcb"][P * c:P * (c + 1), :]),
             "sident": consts["sident"],
             "qrowsTb": np.ascontiguousarray(consts["qcTb"][:, P * c:P * (c + 1)]),
             "sss": consts["sss"],
             "ident": consts["ident"]}
        in_maps.append(m)

    nc = _build()
    r = run_bass_kernel_spmd(nc, in_maps, core_ids=list(range(NC)), trace=trace)
    panels = [r.results[c]["out"] for c in range(NC)]
    full = np.concatenate(panels, axis=1).astype(np.float32)
    return full[None, None], r


def kernel(X, Y):
    out, _ = _run(X, Y, trace=False)
    return out



# revision 27
# speedup vs baseline: 1.0769x; 1.0769x over previous
"""Jacobi 100-step solver on 8 trn2 cores via truncated DST-spectral transform.

v2: the explicit first Jacobi step is folded into mode space. With Qc the
[N, K] interior-DST basis (zero at boundary rows), the masked stencil
satisfies  G1 = Qc^T x1 Qc = s_ab * G0 + 0.25*(rank-1 boundary spikes)
where G0 = Qc^T x0 Qc, s_ab = lam_a + lam_b, and the spikes come from the
boundary rows/cols of x0 (u_top/u_bot via row projections, v_left/v_right
already present as rows of B = x0^T Qc on the edge cores). This removes the
full-field stencil matmuls, halo exchange, and the sign-flip field pass.
x100 = Qc [W99 * G1] Qc^T restricted to the (lo,lo)+(hi,hi) spectral corners
(K=256 per corner). Backward uses the DST checkerboard identity
Qhi = D Qlo to halve the final projection: out_odd = QloT_odd^T (Zlo+Zhi),
out_even = QloT_even^T (Zlo-Zhi), with QloT uploaded parity-packed.

All device I/O is host-prearranged into flat [128, W] blobs so every load is
ONE large DMA trigger striped across all 16 DMA engines (the v1 baseline
spent ~30us issuing ~50 small triggers at ~700ns each on the sync queue).
mm2 produces G^T directly (lhsT = Qrows) so no DMA transposes are needed
after the AllReduce. Everything on-device is fp16 with fp32 PSUM/interm.
"""

import sys
import types
import numpy as np

N = 2048
NC = 8
P = N // NC          # 256 panel columns per core
K = 256              # spectral corner size per corner
RC = N // 128        # 16 row chunks

# cbE blob offsets (fp16 columns)
OF_QROWS = 0         # [128, 4*256] (jm, t) -> Qc_t[256*core + jm*128 + p, b]
OF_QROWSL = 1024     # [128, 4*256] (jm, t) -> Qc_t[...] * lam_t[b]  (b-side scale)
OF_QROWST = 2048     # [128, 4*256] (t, bm) -> Qc_t[256*core + c, bm*128 + p]
OF_W99 = 3072        # [128, 4*256] (t, bm) -> W99_t[bm*128+p, a]
OF_LAMR = 4096       # [128, 4*256] (jm, t) -> lam_t[a] replicated over partitions
OF_SVU = 5120        # row0: svec_lo | svecp_lo | svec_hi | svecp_hi (x0.25)
OF_SVV = 6144        # row0: vleft sel lo|hi, vright sel lo|hi (x0.25, edge cores)
OF_IDENT = 7168      # [128, 128] fp16 identity
CBW = 7296


def _install_ntff_hook():
    if "antenv.axon_hooks" in sys.modules:
        return
    mod = types.ModuleType("antenv.axon_hooks")
    mod._hook = None
    mod.set_axon_ntff_profile_hook = lambda h: setattr(mod, "_hook", h)
    mod.get_axon_ntff_profile_hook = lambda: mod._hook
    sys.modules["antenv.axon_hooks"] = mod
    try:
        import antenv
        antenv.axon_hooks = mod
        from trn_agent_boot.trn_boot import _ntff_profile_via_ctypes
        h = _ntff_profile_via_ctypes("/opt/axon/libaxon_pjrt.so")
        if h is not None:
            mod.set_axon_ntff_profile_hook(h)
    except Exception:
        pass


_HOST_CACHE = {}


def _host_constants():
    if _HOST_CACHE:
        return _HOST_CACHE
    i = np.arange(N, dtype=np.float64)
    Qs, lams, svecs, svecps = [], [], [], []
    for lo in (True, False):
        m = np.arange(1, K + 1, dtype=np.float64) if lo else np.arange(N - 2, N - 2 - K, -1, dtype=np.float64)
        red = np.outer(i, m) % (2 * (N - 1))
        Qc = np.sqrt(2.0 / (N - 1)) * np.sin(np.pi * red / (N - 1))   # [N, K]
        lam = 0.5 * np.cos(np.pi * m / (N - 1))
        Qs.append(Qc)
        lams.append(lam)
        svecs.append(Qc[1, :].copy())
        svecps.append(Qc[N - 2, :].copy())

    # qb: [128, 8192] = [qlo c0-7 | qhi c0-7 | qlo c8-15 | qhi c8-15]
    qb = np.zeros((128, 8192), np.float64)
    for r in range(RC):
        h, rr = r // 8, r % 8
        qb[:, h * 4096 + rr * 256: h * 4096 + rr * 256 + 256] = Qs[0][128 * r:128 * (r + 1), :]
        qb[:, h * 4096 + 2048 + rr * 256: h * 4096 + 2048 + rr * 256 + 256] = Qs[1][128 * r:128 * (r + 1), :]

    # qcT parity packed [128, 4096]: [am, par, ic, j] = Qlo[256*ic + 2*j + par, 128*am + p]
    qcT = np.zeros((128, 4096), np.float64)
    j = np.arange(128)
    for am in range(2):
        for par in range(2):
            for ic in range(8):
                rows = 256 * ic + 2 * j + par
                qcT[:, am * 2048 + par * 1024 + ic * 128: am * 2048 + par * 1024 + (ic + 1) * 128] = \
                    Qs[0][rows, 128 * am:128 * (am + 1)].T

    # w99T [128, 1024]: slot s = 2*t + bm
    w99T = np.zeros((128, 1024), np.float64)
    for t in range(2):
        sab = lams[t][:, None] + lams[t][None, :]      # [b, a] (symmetric)
        w99 = sab ** 99
        for bm in range(2):
            s = 2 * t + bm
            w99T[:, s * 256:(s + 1) * 256] = w99[bm * 128:(bm + 1) * 128, :]

    _HOST_CACHE.update(qb=qb.astype(np.float16), qcT=qcT.astype(np.float16),
                       w99T=w99T, lams=lams, Qs=Qs, svecs=svecs, svecps=svecps)
    return _HOST_CACHE


def _core_cbE(c, hc):
    """Per-core consts blob [128, CBW] fp16."""
    Qs, svecs, svecps, lams = hc["Qs"], hc["svecs"], hc["svecps"], hc["lams"]
    cb = np.zeros((128, CBW), np.float64)
    # qrowsb / qrowsLb / lamrow: (jm, t)
    for jm in range(2):
        for t in range(2):
            s = 2 * jm + t
            rows = Qs[t][256 * c + 128 * jm: 256 * c + 128 * (jm + 1), :]
            cb[:, OF_QROWS + s * 256: OF_QROWS + (s + 1) * 256] = rows
            cb[:, OF_QROWSL + s * 256: OF_QROWSL + (s + 1) * 256] = rows * lams[t][None, :]
            cb[:, OF_LAMR + s * 256: OF_LAMR + (s + 1) * 256] = lams[t][None, :]
    # qrowsTb: (t, bm)
    for t in range(2):
        for bm in range(2):
            s = 2 * t + bm
            cb[:, OF_QROWST + s * 256: OF_QROWST + (s + 1) * 256] = \
                Qs[t][256 * c: 256 * (c + 1), 128 * bm: 128 * (bm + 1)].T
    cb[:, OF_W99: OF_W99 + 1024] = hc["w99T"]
    # svu (all cores): svec_lo | svecp_lo | svec_hi | svecp_hi, x0.25
    for t in range(2):
        cb[0, OF_SVU + t * 512: OF_SVU + t * 512 + 256] = 0.25 * svecs[t]
        cb[0, OF_SVU + t * 512 + 256: OF_SVU + t * 512 + 512] = 0.25 * svecps[t]
    # svv: vleft sel (core 0): svec_lo | svec_hi ; vright sel (core 7): svecp_lo | svecp_hi
    if c == 0:
        for t in range(2):
            cb[0, OF_SVV + t * 256: OF_SVV + (t + 1) * 256] = 0.25 * svecs[t]
    if c == NC - 1:
        for t in range(2):
            cb[0, OF_SVV + 512 + t * 256: OF_SVV + 512 + (t + 1) * 256] = 0.25 * svecps[t]
    cb[:, OF_IDENT: OF_IDENT + 128] = np.eye(128)
    return cb.astype(np.float16)


_NC_CACHE = {}


def _build():
    if "nc" in _NC_CACHE:
        return _NC_CACHE["nc"]
    import concourse.bacc as bacc
    import concourse.tile as tile
    import concourse.mybir as mybir

    F16 = mybir.dt.float16
    F32 = mybir.dt.float32
    ALU = mybir.AluOpType
    ACTF = mybir.ActivationFunctionType
    nc = bacc.Bacc("TRN2", target_bir_lowering=False, debug=False, num_devices=NC)

    xyb_d = nc.dram_tensor("xyb", [128, 8192], F16, kind="ExternalInput")
    qb_d = nc.dram_tensor("qb", [128, 8192], F16, kind="ExternalInput")
    cbE_d = nc.dram_tensor("cbE", [128, CBW], F16, kind="ExternalInput")
    qcT_d = nc.dram_tensor("qcT", [128, 4096], F16, kind="ExternalInput")
    out_d = nc.dram_tensor("out", [128, 4096], F16, kind="ExternalOutput")

    with tile.TileContext(nc) as tc:
        with tc.tile_pool(name="pers", bufs=1) as pers, \
             tc.tile_pool(name="ps", bufs=1, space="PSUM") as ps, \
             tc.tile_pool(name="dram", bufs=2, space="DRAM") as dram:

            # ---- persistent SBUF ----
            xyb_s = pers.tile([128, 8192], F16, tag="xyb")
            qb_s = pers.tile([128, 8192], F16, tag="qb")
            cbE_s = pers.tile([128, CBW], F16, tag="cbE")
            qcT_s = pers.tile([128, 4096], F16, tag="qcT")
            x0b = pers.tile([128, 4096], F16, tag="x0b")
            t2b = pers.tile([128, 4096], F32, tag="t2b")
            t3b = pers.tile([128, 4096], F32, tag="t3b")
            d2b = pers.tile([128, 4096], F32, tag="d2b")
            abuf = pers.tile([128, 1024], F16, tag="abuf")
            abufL = pers.tile([128, 1024], F16, tag="abufL")
            usb = pers.tile([128, 1024], F16, tag="usb")
            x0rT = pers.tile([128, 4], F16, tag="x0rT")
            vrow = pers.tile([128, 512], F16, tag="vrow")
            gsb = pers.tile([128, 1024], F16, tag="gsb")
            gout_s = pers.tile([128, 1024], F16, tag="gouts")
            utb = pers.tile([128, 1024], F16, tag="utb")
            zbuf = pers.tile([128, 1024], F16, tag="zbuf")
            ztmp = pers.tile([128, 512], F16, tag="ztmp")
            outb = pers.tile([128, 4096], F16, tag="outb")
            zt = pers.tile([128, 1], F32, tag="zt")

            # const APs for activation biases
            cexp = pers.tile([128, 1], F32, tag="cexp", name="cexp")
            nc.vector.memset(cexp[:], -12.5)
            nc.const_aps.aps[(F32, -12.5)] = cexp[:]
            csq = pers.tile([128, 1], F32, tag="csq", name="csq")
            nc.vector.memset(csq[:], -0.5)
            nc.const_aps.aps[(F32, -0.5)] = csq[:]

            # ---- warmup barrier AllReduce on the CC engine ----
            barrier_in = dram.tile([128, 1], F32, tag="barin")
            barrier_out = dram.tile([128, 1], F32, tag="barout", addr_space="Shared")
            nc.vector.memset(zt[:], 0.0)
            nc.scalar.dma_start(barrier_in[:, :], zt[:])
            nc.gpsimd.collective_compute(
                "AllReduce", ALU.add,
                replica_groups=[list(range(NC))],
                ins=[barrier_in.opt()], outs=[barrier_out.opt()],
            )

            # ---- critical loads: strict FIFO order on the sync ring ----
            nc.sync.dma_start(xyb_s[:, 0:4096], xyb_d[:, 0:4096])
            nc.sync.dma_start(qb_s[:, 0:4096], qb_d[:, 0:4096])
            nc.sync.dma_start(xyb_s[:, 4096:8192], xyb_d[:, 4096:8192])
            nc.sync.dma_start(qb_s[:, 4096:8192], qb_d[:, 4096:8192])
            nc.sync.dma_start(cbE_s[:], cbE_d[:, :])
            nc.sync.dma_start(qcT_s[:], qcT_d[:, :])

            ident_s = cbE_s[:, OF_IDENT:OF_IDENT + 128]

            # ---- A accumulators: [c-part(jm), modes] x (lo, hi) ----
            aps = [ps.tile([128, K], F32, tag="aacc", bufs=4, name=f"aps{j}") for j in range(4)]
            # slot j = 2*jm + t

            def qslot(r, t):
                h, rr = r // 8, r % 8
                return qb_s[:, h * 4096 + t * 2048 + rr * 256: h * 4096 + t * 2048 + (rr + 1) * 256]

            def mm1_chunk(r):
                for jm in range(2):
                    lhsT = x0b[:, r * 256 + jm * 128: r * 256 + (jm + 1) * 128]
                    for t in range(2):
                        nc.tensor.matmul(aps[2 * jm + t][:], lhsT, qslot(r, t),
                                         start=(r == 0), stop=(r == RC - 1))

            def fwd_block(blk):
                sl = slice(blk * 1024, (blk + 1) * 1024)
                h, b2 = blk // 2, blk % 2
                xs = xyb_s[:, h * 4096 + b2 * 1024: h * 4096 + (b2 + 1) * 1024]
                ys = xyb_s[:, h * 4096 + 2048 + b2 * 1024: h * 4096 + 2048 + (b2 + 1) * 1024]
                # d2' = X(X-1) + (Y-.5)^2 = d^2 - 0.25
                nc.vector.scalar_tensor_tensor(t2b[:, sl], xs, -1.0, xs, ALU.add, ALU.mult)
                nc.scalar.activation(t3b[:, sl], ys, ACTF.Square, bias=-0.5, scale=1.0)
                nc.gpsimd.tensor_add(d2b[:, sl], t2b[:, sl], t3b[:, sl])
                # x0 = exp(-50*d^2) = exp(-50*d2' - 12.5)
                nc.scalar.activation(x0b[:, sl], d2b[:, sl], ACTF.Exp, bias=-12.5, scale=-50.0)
                for r in range(4 * blk, 4 * blk + 4):
                    mm1_chunk(r)

            def row_transpose(col_lo, out_col, take_row):
                # x0b[:, col_lo:col_lo+128]^T -> psum; column take_row = field row
                tp = ps.tile([128, 128], F16, tag="pp", bufs=4, name="pp")
                nc.tensor.transpose(tp[:], x0b[:, col_lo:col_lo + 128], ident_s)
                nc.vector.tensor_copy(x0rT[:, out_col:out_col + 1], tp[:, take_row:take_row + 1])

            def u_project(xcol0, xcol1, dst_off):
                # u_t = x0row^T . qrows_t for both corners -> usb row 0
                for t in range(2):
                    ups = ps.tile([128, K], F32, tag="pp", bufs=4, name="pp")
                    nc.tensor.matmul(ups[0:1, :], x0rT[:, xcol0:xcol0 + 1],
                                     cbE_s[:, OF_QROWS + t * 256: OF_QROWS + (t + 1) * 256],
                                     start=True, stop=False)
                    nc.tensor.matmul(ups[0:1, :], x0rT[:, xcol1:xcol1 + 1],
                                     cbE_s[:, OF_QROWS + (2 + t) * 256: OF_QROWS + (3 + t) * 256],
                                     start=False, stop=True)
                    nc.vector.tensor_copy(usb[0:1, dst_off + t * 256: dst_off + (t + 1) * 256], ups[0:1, :])

            # forward blocks; u_top machinery after blk0, u_bot after blk3
            fwd_block(0)
            row_transpose(0, 0, 0)          # x0 row 0, cols 0-127
            row_transpose(128, 1, 0)        # x0 row 0, cols 128-255
            u_project(0, 1, 0)              # usb[0:512] = u_top (lo|hi)
            fwd_block(1)
            fwd_block(2)
            fwd_block(3)
            row_transpose(15 * 256, 2, 127)       # x0 row 2047, cols 0-127
            row_transpose(15 * 256 + 128, 3, 127)
            u_project(2, 3, 512)            # usb[512:1024] = u_bot (lo|hi)

            # ---- drain A -> abuf (fp16) and abufL = A * lam_a (for the a-side of
            # sab*G0 = (Qrows L)^T B + Qrows^T (B L), merged into one psum group) ----
            for j in range(4):
                jm, t = j // 2, j % 2
                sl = slice(jm * 512 + t * 256, jm * 512 + (t + 1) * 256)
                nc.scalar.copy(abuf[:, sl], aps[j][:])
                nc.vector.tensor_mul(abufL[:, sl], aps[j][:], cbE_s[:, OF_LAMR + j * 256: OF_LAMR + (j + 1) * 256])

            # v_right row (panel col 255) lives at abuf partition 127; PE operands
            # must start at partition 0 -> stage it down via SBUF-to-SBUF DMA
            nc.scalar.dma_start(vrow[0:1, :], abuf[127:128, 512:1024])

            # ---- mm2: G1^T = (Qrows L)^T B + Qrows^T (B L) + rank-1 boundary
            # spikes, all in one psum accumulation group per (t, bm) ----
            gin = dram.tile([128, 1024], F16, tag="gin")
            gout = dram.tile([128, 1024], F16, tag="gout", addr_space="Shared")
            for t in range(2):
                for bm in range(2):
                    s = 2 * t + bm
                    gp = ps.tile([128, K], F32, tag="pp", bufs=4, name="pp")
                    for jm in range(2):
                        qsl = slice(OF_QROWS + (2 * jm + t) * 256 + bm * 128,
                                    OF_QROWS + (2 * jm + t) * 256 + (bm + 1) * 128)
                        qLsl = slice(OF_QROWSL + (2 * jm + t) * 256 + bm * 128,
                                     OF_QROWSL + (2 * jm + t) * 256 + (bm + 1) * 128)
                        asl = slice(jm * 512 + t * 256, jm * 512 + (t + 1) * 256)
                        nc.tensor.matmul(gp[:], cbE_s[:, qLsl], abuf[:, asl],
                                         start=(jm == 0), stop=False)
                        nc.tensor.matmul(gp[:], cbE_s[:, qsl], abufL[:, asl],
                                         start=False, stop=False)
                    nc.tensor.matmul(gp[:], usb[0:1, t * 256 + bm * 128: t * 256 + (bm + 1) * 128],
                                     cbE_s[0:1, OF_SVU + t * 512: OF_SVU + t * 512 + 256],
                                     start=False, stop=False)
                    nc.tensor.matmul(gp[:], usb[0:1, 512 + t * 256 + bm * 128: 512 + t * 256 + (bm + 1) * 128],
                                     cbE_s[0:1, OF_SVU + t * 512 + 256: OF_SVU + t * 512 + 512],
                                     start=False, stop=False)
                    nc.tensor.matmul(gp[:], cbE_s[0:1, OF_SVV + t * 256 + bm * 128: OF_SVV + t * 256 + (bm + 1) * 128],
                                     abuf[0:1, t * 256: (t + 1) * 256],
                                     start=False, stop=False)
                    nc.tensor.matmul(gp[:], cbE_s[0:1, OF_SVV + 512 + t * 256 + bm * 128: OF_SVV + 512 + t * 256 + (bm + 1) * 128],
                                     vrow[0:1, t * 256: (t + 1) * 256],
                                     start=False, stop=True)
                    if s % 2 == 0:
                        nc.vector.tensor_copy(gsb[:, s * 256:(s + 1) * 256], gp[:])
                    else:
                        nc.scalar.copy(gsb[:, s * 256:(s + 1) * 256], gp[:])
            nc.sync.dma_start(gin[:, :], gsb[:])
            nc.gpsimd.collective_compute(
                "AllReduce", ALU.add,
                replica_groups=[list(range(NC))],
                ins=[gin.opt()], outs=[gout.opt()],
            )
            nc.sync.dma_start(gout_s[:], gout[:, :])

            # ---- filter: U^T = W99 * G1^T (SBUF only: vector+gpsimd ok) ----
            engs = (nc.vector, nc.gpsimd)
            for s in range(4):
                sl = slice(s * 256, (s + 1) * 256)
                engs[s % 2].tensor_mul(utb[:, sl], gout_s[:, sl],
                                       cbE_s[:, OF_W99 + s * 256: OF_W99 + (s + 1) * 256])

            # ---- B1: Z_t[a, c] = sum_b U_t[b, a] Qrows_t[c, b] ----
            # Zp (odd rows) at zbuf[0:512], Zm (even rows) at zbuf[512:1024]
            for am in range(2):
                zps = []
                for t in range(2):
                    zp = ps.tile([128, K], F32, tag="pp", bufs=4, name="pp")
                    for bm in range(2):
                        s = 2 * t + bm
                        nc.tensor.matmul(zp[:],
                                         utb[:, s * 256 + am * 128: s * 256 + (am + 1) * 128],
                                         cbE_s[:, OF_QROWST + s * 256: OF_QROWST + (s + 1) * 256],
                                         start=(bm == 0), stop=(bm == 1))
                    zps.append(zp)
                # DVE can only read one PSUM operand per op: stage Z_lo first
                zsl = slice(am * 256, (am + 1) * 256)
                nc.scalar.copy(ztmp[:, zsl], zps[0][:])
                nc.vector.tensor_add(zbuf[:, am * 256: (am + 1) * 256], ztmp[:, zsl], zps[1][:])
                nc.vector.tensor_sub(zbuf[:, 512 + am * 256: 512 + (am + 1) * 256], ztmp[:, zsl], zps[1][:])

            # ---- B2: out chunks via parity-packed QloT ----
            dr_engs = (nc.scalar, nc.vector)
            di = 0
            for ic in range(8):
                for par in range(2):
                    ops = ps.tile([128, K], F32, tag="pp", bufs=4, name="pp")
                    zoff = 0 if par == 1 else 512
                    for am in range(2):
                        nc.tensor.matmul(ops[:],
                                         qcT_s[:, am * 2048 + par * 1024 + ic * 128: am * 2048 + par * 1024 + (ic + 1) * 128],
                                         zbuf[:, zoff + am * 256: zoff + (am + 1) * 256],
                                         start=(am == 0), stop=(am == 1))
                    dst = outb[:, (2 * ic + par) * 256: (2 * ic + par + 1) * 256]
                    eng = dr_engs[di % 2]; di += 1
                    if eng is nc.scalar:
                        eng.copy(dst, ops[:])
                    else:
                        eng.tensor_copy(dst, ops[:])
                if ic == 3:
                    nc.scalar.dma_start(out_d[:, 0:2048], outb[:, 0:2048])
            nc.scalar.dma_start(out_d[:, 2048:4096], outb[:, 2048:4096])

    nc.compile()
    _NC_CACHE["nc"] = nc
    return nc


def _run(X, Y, trace=False):
    _install_ntff_hook()
    from concourse.bass_utils import run_bass_kernel_spmd

    hc = _host_constants()
    Xf = np.asarray(X, np.float32).astype(np.float16)
    Yf = np.asarray(Y, np.float32).astype(np.float16)

    in_maps = []
    for c in range(NC):
        xp = Xf[:, P * c: P * (c + 1)]           # [2048, 256]
        yp = Yf[:, P * c: P * (c + 1)]
        xyb = np.zeros((128, 8192), np.float16)
        for r in range(RC):
            h, rr = r // 8, r % 8
            xyb[:, h * 4096 + rr * 256: h * 4096 + (rr + 1) * 256] = xp[128 * r:128 * (r + 1), :]
            xyb[:, h * 4096 + 2048 + rr * 256: h * 4096 + 2048 + (rr + 1) * 256] = yp[128 * r:128 * (r + 1), :]
        m = {"xyb": xyb,
             "qb": hc["qb"],
             "cbE": _core_cbE(c, hc),
             "qcT": hc["qcT"]}
        in_maps.append(m)

    nc = _build()
    r = run_bass_kernel_spmd(nc, in_maps, core_ids=list(range(NC)), trace=trace)
    panels = []
    for c in range(NC):
        o = r.results[c]["out"].reshape(128, 8, 2, 256)      # [p, ic, par, c]
        panels.append(o.transpose(1, 0, 2, 3).reshape(2048, 256))
    full = np.concatenate(panels, axis=1).astype(np.float32)
    return full[None, None], r


def kernel(X, Y):
    out, _ = _run(X, Y, trace=False)
    return out


# revision 28
# speedup vs baseline: 1.1207x; 1.0407x over previous
"""Jacobi 100-step solver on 8 trn2 cores via truncated DST-spectral transform.

v3: the explicit first Jacobi step is folded into mode space. With Qc the
[N, K] interior-DST basis (zero at boundary rows), the masked stencil
satisfies  G1 = Qc^T x1 Qc = s_ab * G0 + 0.25*(rank-1 boundary spikes)
where G0 = Qc^T x0 Qc, s_ab = lam_a + lam_b; the spikes come from the
boundary rows/cols of x0 (u_top/u_bot via PE row transposes + projections,
v_left/v_right are rows of B = x0^T Qc on the edge cores, gated by per-core
zeroed spike vectors). The s_ab multiply rides the matmuls:
sab*G0 = (Qrows*lam)^T B + Qrows^T (B*lam), so spikes share the same PSUM
accumulation and the drain is a plain copy. x100 = Qc [W99 * G1] Qc^T on the
(lo,lo)+(hi,hi) corners (K=256 each). Backward halves via the checkerboard
identity Qhi = D Qlo: out_odd = QloT_odd^T (Zlo+Zhi), out_even with (Zlo-Zhi),
QloT uploaded parity-packed. The hi-corner forward uses x0d = D*x0 (sign per
partition) against the same qlo, so only one basis panel is uploaded.

All device I/O is host-prearranged into flat [128, W] blobs -> each load is
ONE DMA trigger striped across all 16 DMA engines. Trigger order on the sync
ring prioritizes the forward-critical bytes; backward consts stream in the
AllReduce window. A zero-payload warmup AllReduce is doorbelled at ~10us so
the NEFF collective barrier + first-op setup cost burns during forward; the
real [128,1024] fp16 AllReduce then chains with ~2us latency.
"""

import sys
import types
import numpy as np

N = 2048
NC = 8
P = N // NC          # 256 panel columns per core
K = 256              # spectral corner size per corner
RC = N // 128        # 16 row chunks

# cbQ blob offsets (forward-critical consts)
OF_QROWS = 0         # [128, 4*256] (jm, t) -> Qc_t[256*core + jm*128 + p, b]
OF_QROWSL = 1024     # same * lam_t[b]
OF_LAMR = 2048       # [128, 4*256] (jm, t) -> lam_t[a] replicated over partitions
OF_SVU = 3072        # row0: svec_lo | svecp_lo | svec_hi | svecp_hi (x0.25)
OF_SVV = 4096        # row0: vleft sel lo|hi, vright sel lo|hi (x0.25, edge cores)
OF_IDENT = 5120      # [128, 128] fp16 identity
CBQW = 5248
# cbB blob offsets (backward consts, stream during AllReduce)
OF_QROWST = 0        # [128, 4*256] (t, bm) -> Qc_t[256*core + c, bm*128 + p]
OF_W99 = 1024        # [128, 4*256] (t, bm) -> W99_t[bm*128+p, a]
CBBW = 2048


def _install_ntff_hook():
    if "antenv.axon_hooks" in sys.modules:
        return
    mod = types.ModuleType("antenv.axon_hooks")
    mod._hook = None
    mod.set_axon_ntff_profile_hook = lambda h: setattr(mod, "_hook", h)
    mod.get_axon_ntff_profile_hook = lambda: mod._hook
    sys.modules["antenv.axon_hooks"] = mod
    try:
        import antenv
        antenv.axon_hooks = mod
        from trn_agent_boot.trn_boot import _ntff_profile_via_ctypes
        h = _ntff_profile_via_ctypes("/opt/axon/libaxon_pjrt.so")
        if h is not None:
            mod.set_axon_ntff_profile_hook(h)
    except Exception:
        pass


_HOST_CACHE = {}


def _host_constants():
    if _HOST_CACHE:
        return _HOST_CACHE
    i = np.arange(N, dtype=np.float64)
    Qs, lams, svecs, svecps = [], [], [], []
    for lo in (True, False):
        m = np.arange(1, K + 1, dtype=np.float64) if lo else np.arange(N - 2, N - 2 - K, -1, dtype=np.float64)
        red = np.outer(i, m) % (2 * (N - 1))
        Qc = np.sqrt(2.0 / (N - 1)) * np.sin(np.pi * red / (N - 1))   # [N, K]
        lam = 0.5 * np.cos(np.pi * m / (N - 1))
        Qs.append(Qc)
        lams.append(lam)
        svecs.append(Qc[1, :].copy())
        svecps.append(Qc[N - 2, :].copy())

    # qb: [128, 4096] = qlo chunks (halves): slot(r) = (r//8)*2048 + (r%8)*256
    qb = np.zeros((128, 4096), np.float64)
    for r in range(RC):
        h, rr = r // 8, r % 8
        qb[:, h * 2048 + rr * 256: h * 2048 + (rr + 1) * 256] = Qs[0][128 * r:128 * (r + 1), :]

    # qcT parity packed [128, 4096]: [am, par, ic, j] = Qlo[256*ic + 2*j + par, 128*am + p]
    qcT = np.zeros((128, 4096), np.float64)
    j = np.arange(128)
    for am in range(2):
        for par in range(2):
            for ic in range(8):
                rows = 256 * ic + 2 * j + par
                qcT[:, am * 2048 + par * 1024 + ic * 128: am * 2048 + par * 1024 + (ic + 1) * 128] = \
                    Qs[0][rows, 128 * am:128 * (am + 1)].T

    # w99T [128, 1024]: slot s = 2*t + bm
    w99T = np.zeros((128, 1024), np.float64)
    for t in range(2):
        sab = lams[t][:, None] + lams[t][None, :]      # [b, a] (symmetric)
        w99 = sab ** 99
        for bm in range(2):
            s = 2 * t + bm
            w99T[:, s * 256:(s + 1) * 256] = w99[bm * 128:(bm + 1) * 128, :]

    _HOST_CACHE.update(qb=qb.astype(np.float16), qcT=qcT.astype(np.float16),
                       w99T=w99T, lams=lams, Qs=Qs, svecs=svecs, svecps=svecps)
    return _HOST_CACHE


def _core_cbQ(c, hc):
    Qs, svecs, svecps, lams = hc["Qs"], hc["svecs"], hc["svecps"], hc["lams"]
    cb = np.zeros((128, CBQW), np.float64)
    for jm in range(2):
        for t in range(2):
            s = 2 * jm + t
            rows = Qs[t][256 * c + 128 * jm: 256 * c + 128 * (jm + 1), :]
            cb[:, OF_QROWS + s * 256: OF_QROWS + (s + 1) * 256] = rows
            cb[:, OF_QROWSL + s * 256: OF_QROWSL + (s + 1) * 256] = rows * lams[t][None, :]
            cb[:, OF_LAMR + s * 256: OF_LAMR + (s + 1) * 256] = lams[t][None, :]
    for t in range(2):
        cb[0, OF_SVU + t * 512: OF_SVU + t * 512 + 256] = 0.25 * svecs[t]
        cb[0, OF_SVU + t * 512 + 256: OF_SVU + t * 512 + 512] = 0.25 * svecps[t]
    if c == 0:
        for t in range(2):
            cb[0, OF_SVV + t * 256: OF_SVV + (t + 1) * 256] = 0.25 * svecs[t]
    if c == NC - 1:
        for t in range(2):
            cb[0, OF_SVV + 512 + t * 256: OF_SVV + 512 + (t + 1) * 256] = 0.25 * svecps[t]
    cb[:, OF_IDENT: OF_IDENT + 128] = np.eye(128)
    return cb.astype(np.float16)


def _core_cbB(c, hc):
    Qs = hc["Qs"]
    cb = np.zeros((128, CBBW), np.float64)
    for t in range(2):
        for bm in range(2):
            s = 2 * t + bm
            cb[:, OF_QROWST + s * 256: OF_QROWST + (s + 1) * 256] = \
                Qs[t][256 * c: 256 * (c + 1), 128 * bm: 128 * (bm + 1)].T
    cb[:, OF_W99: OF_W99 + 1024] = hc["w99T"]
    return cb.astype(np.float16)


_NC_CACHE = {}


def _build():
    if "nc" in _NC_CACHE:
        return _NC_CACHE["nc"]
    import concourse.bacc as bacc
    import concourse.tile as tile
    import concourse.mybir as mybir

    F16 = mybir.dt.float16
    F32 = mybir.dt.float32
    ALU = mybir.AluOpType
    ACTF = mybir.ActivationFunctionType
    nc = bacc.Bacc("TRN2", target_bir_lowering=False, debug=False, num_devices=NC)

    # xyb col 0 = sgncol ((-1)^(p+1)); then [x 0-7 | y 0-7 | x 8-15 | y 8-15]
    xyb_d = nc.dram_tensor("xyb", [128, 8193], F16, kind="ExternalInput")
    qb_d = nc.dram_tensor("qb", [128, 4096], F16, kind="ExternalInput")
    cbQ_d = nc.dram_tensor("cbQ", [128, CBQW], F16, kind="ExternalInput")
    cbB_d = nc.dram_tensor("cbB", [128, CBBW], F16, kind="ExternalInput")
    qcT_d = nc.dram_tensor("qcT", [128, 4096], F16, kind="ExternalInput")
    out_d = nc.dram_tensor("out", [128, 4096], F16, kind="ExternalOutput")

    with tile.TileContext(nc) as tc:
        with tc.tile_pool(name="pers", bufs=1) as pers, \
             tc.tile_pool(name="ps", bufs=1, space="PSUM") as ps, \
             tc.tile_pool(name="dram", bufs=1, space="DRAM") as dram:

            # ---- persistent SBUF ----
            xyb_s = pers.tile([128, 8193], F16, tag="xyb")
            qb_s = pers.tile([128, 4096], F16, tag="qb")
            cbQ_s = pers.tile([128, CBQW], F16, tag="cbQ")
            cbB_s = pers.tile([128, CBBW], F16, tag="cbB")
            qcT_s = pers.tile([128, 4096], F16, tag="qcT")
            x0b = pers.tile([128, 4096], F16, tag="x0b")
            x0d = pers.tile([128, 4096], F16, tag="x0d")
            t2b = pers.tile([128, 4096], F32, tag="t2b")
            t3b = pers.tile([128, 4096], F32, tag="t3b")
            d2b = pers.tile([128, 4096], F32, tag="d2b")
            abuf = pers.tile([128, 1024], F16, tag="abuf")
            abufL = pers.tile([128, 1024], F16, tag="abufL")
            usb = pers.tile([128, 1024], F16, tag="usb")
            x0rT = pers.tile([128, 4], F16, tag="x0rT")
            vrow = pers.tile([128, 512], F16, tag="vrow")
            gsb = pers.tile([128, 1024], F16, tag="gsb")
            gout_s = pers.tile([128, 1024], F16, tag="gouts")
            utb = pers.tile([128, 1024], F16, tag="utb")
            zbuf = pers.tile([128, 1024], F16, tag="zbuf")
            ztmp = pers.tile([128, 512], F16, tag="ztmp")
            outb = pers.tile([128, 4096], F16, tag="outb")
            zt = pers.tile([128, 1], F32, tag="zt")

            # const APs for activation biases
            cexp = pers.tile([128, 1], F32, tag="cexp", name="cexp")
            nc.vector.memset(cexp[:], -12.5)
            nc.const_aps.aps[(F32, -12.5)] = cexp[:]
            csq = pers.tile([128, 1], F32, tag="csq", name="csq")
            nc.vector.memset(csq[:], -0.5)
            nc.const_aps.aps[(F32, -0.5)] = csq[:]

            # ---- warmup AllReduce: dedicated dram tiles (no pool aliasing),
            # doorbell on gpsimd as early as possible ----
            barrier_in = dram.tile([128, 1], F32, tag="barin")
            barrier_out = dram.tile([128, 1], F32, tag="barout", addr_space="Shared")
            nc.vector.memset(zt[:], 0.0)
            nc.scalar.dma_start(barrier_in[:, :], zt[:])
            nc.gpsimd.collective_compute(
                "AllReduce", ALU.add,
                replica_groups=[list(range(NC))],
                ins=[barrier_in.opt()], outs=[barrier_out.opt()],
            )

            # ---- critical loads: strict FIFO order on the sync ring ----
            nc.sync.dma_start(xyb_s[:, 0:4097], xyb_d[:, 0:4097])       # sgncol + xA|yA
            nc.sync.dma_start(qb_s[:, 0:2048], qb_d[:, 0:2048])         # qlo 0-7
            nc.sync.dma_start(xyb_s[:, 4097:8193], xyb_d[:, 4097:8193])  # xB|yB
            nc.sync.dma_start(qb_s[:, 2048:4096], qb_d[:, 2048:4096])   # qlo 8-15
            nc.sync.dma_start(cbQ_s[:], cbQ_d[:, :])
            nc.sync.dma_start(cbB_s[:], cbB_d[:, :])
            nc.sync.dma_start(qcT_s[:], qcT_d[:, :])

            sgncol = xyb_s[:, 0:1]
            ident_s = cbQ_s[:, OF_IDENT:OF_IDENT + 128]

            # ---- A accumulators: [c-part(jm), modes] x (lo, hi) ----
            aps = [ps.tile([128, K], F32, tag="aacc", bufs=4, name=f"aps{j}") for j in range(4)]
            # slot j = 2*jm + t

            def qslot(r):
                h, rr = r // 8, r % 8
                return qb_s[:, h * 2048 + rr * 256: h * 2048 + (rr + 1) * 256]

            def mm1_chunk(r):
                for jm in range(2):
                    sl = slice(r * 256 + jm * 128, r * 256 + (jm + 1) * 128)
                    nc.tensor.matmul(aps[2 * jm][:], x0b[:, sl], qslot(r),
                                     start=(r == 0), stop=(r == RC - 1))
                    nc.tensor.matmul(aps[2 * jm + 1][:], x0d[:, sl], qslot(r),
                                     start=(r == 0), stop=(r == RC - 1))

            def fwd_block(blk):
                sl = slice(blk * 1024, (blk + 1) * 1024)
                h, b2 = blk // 2, blk % 2
                xs = xyb_s[:, 1 + h * 4096 + b2 * 1024: 1 + h * 4096 + (b2 + 1) * 1024]
                ys = xyb_s[:, 1 + h * 4096 + 2048 + b2 * 1024: 1 + h * 4096 + 2048 + (b2 + 1) * 1024]
                # d2' = X(X-1) + (Y-.5)^2 = d^2 - 0.25
                nc.vector.scalar_tensor_tensor(t2b[:, sl], xs, -1.0, xs, ALU.add, ALU.mult)
                nc.scalar.activation(t3b[:, sl], ys, ACTF.Square, bias=-0.5, scale=1.0)
                nc.vector.tensor_add(d2b[:, sl], t2b[:, sl], t3b[:, sl])
                # x0 = exp(-50*d^2) = exp(-50*d2' - 12.5)
                nc.scalar.activation(x0b[:, sl], d2b[:, sl], ACTF.Exp, bias=-12.5, scale=-50.0)
                # hi-corner operand: x0d = D x0 (checkerboard row sign)
                nc.gpsimd.tensor_mul(x0d[:, sl], x0b[:, sl], sgncol.to_broadcast((128, 1024)))
                for r in range(4 * blk, 4 * blk + 4):
                    mm1_chunk(r)

            for blk in range(4):
                fwd_block(blk)

            # ---- boundary row projections (after all mm1 so the PE stream
            # never stalls on cbQ): u_t = x0row . qrows_t ----
            def row_transpose(col_lo, out_col, take_row):
                tp = ps.tile([128, 128], F16, tag="pp", bufs=4, name="pp")
                nc.tensor.transpose(tp[:], x0b[:, col_lo:col_lo + 128], ident_s)
                nc.vector.tensor_copy(x0rT[:, out_col:out_col + 1], tp[:, take_row:take_row + 1])

            def u_project(xcol0, xcol1, dst_off):
                for t in range(2):
                    ups = ps.tile([128, K], F32, tag="pp", bufs=4, name="pp")
                    nc.tensor.matmul(ups[0:1, :], x0rT[:, xcol0:xcol0 + 1],
                                     cbQ_s[:, OF_QROWS + t * 256: OF_QROWS + (t + 1) * 256],
                                     start=True, stop=False)
                    nc.tensor.matmul(ups[0:1, :], x0rT[:, xcol1:xcol1 + 1],
                                     cbQ_s[:, OF_QROWS + (2 + t) * 256: OF_QROWS + (3 + t) * 256],
                                     start=False, stop=True)
                    nc.vector.tensor_copy(usb[0:1, dst_off + t * 256: dst_off + (t + 1) * 256], ups[0:1, :])

            row_transpose(0, 0, 0)                 # x0 row 0, cols 0-127
            row_transpose(128, 1, 0)               # x0 row 0, cols 128-255
            row_transpose(15 * 256, 2, 127)        # x0 row 2047, cols 0-127
            row_transpose(15 * 256 + 128, 3, 127)
            u_project(0, 1, 0)                     # usb[0:512] = u_top (lo|hi)
            u_project(2, 3, 512)                   # usb[512:1024] = u_bot (lo|hi)

            # ---- drain A -> abuf and abufL = A * lam_a ----
            for j in range(4):
                jm, t = j // 2, j % 2
                sl = slice(jm * 512 + t * 256, jm * 512 + (t + 1) * 256)
                nc.scalar.copy(abuf[:, sl], aps[j][:])
                nc.vector.tensor_mul(abufL[:, sl], aps[j][:], cbQ_s[:, OF_LAMR + j * 256: OF_LAMR + (j + 1) * 256])

            # v_right row (panel col 255) lives at abuf partition 127; PE operands
            # must start at partition 0 -> stage it down via SBUF-to-SBUF DMA
            nc.scalar.dma_start(vrow[0:1, :], abuf[127:128, 512:1024])

            # ---- mm2: G1^T = (Qrows L)^T B + Qrows^T (B L) + rank-1 boundary
            # spikes, one psum accumulation group per (t, bm) ----
            gin = dram.tile([128, 1024], F16, tag="gin")
            gout = dram.tile([128, 1024], F16, tag="gout", addr_space="Shared")
            for t in range(2):
                for bm in range(2):
                    s = 2 * t + bm
                    gp = ps.tile([128, K], F32, tag="pp", bufs=4, name="pp")
                    for jm in range(2):
                        qsl = slice(OF_QROWS + (2 * jm + t) * 256 + bm * 128,
                                    OF_QROWS + (2 * jm + t) * 256 + (bm + 1) * 128)
                        qLsl = slice(OF_QROWSL + (2 * jm + t) * 256 + bm * 128,
                                     OF_QROWSL + (2 * jm + t) * 256 + (bm + 1) * 128)
                        asl = slice(jm * 512 + t * 256, jm * 512 + (t + 1) * 256)
                        nc.tensor.matmul(gp[:], cbQ_s[:, qLsl], abuf[:, asl],
                                         start=(jm == 0), stop=False)
                        nc.tensor.matmul(gp[:], cbQ_s[:, qsl], abufL[:, asl],
                                         start=False, stop=False)
                    nc.tensor.matmul(gp[:], usb[0:1, t * 256 + bm * 128: t * 256 + (bm + 1) * 128],
                                     cbQ_s[0:1, OF_SVU + t * 512: OF_SVU + t * 512 + 256],
                                     start=False, stop=False)
                    nc.tensor.matmul(gp[:], usb[0:1, 512 + t * 256 + bm * 128: 512 + t * 256 + (bm + 1) * 128],
                                     cbQ_s[0:1, OF_SVU + t * 512 + 256: OF_SVU + t * 512 + 512],
                                     start=False, stop=False)
                    nc.tensor.matmul(gp[:], cbQ_s[0:1, OF_SVV + t * 256 + bm * 128: OF_SVV + t * 256 + (bm + 1) * 128],
                                     abuf[0:1, t * 256: (t + 1) * 256],
                                     start=False, stop=False)
                    nc.tensor.matmul(gp[:], cbQ_s[0:1, OF_SVV + 512 + t * 256 + bm * 128: OF_SVV + 512 + t * 256 + (bm + 1) * 128],
                                     vrow[0:1, t * 256: (t + 1) * 256],
                                     start=False, stop=True)
                    if s % 2 == 0:
                        nc.vector.tensor_copy(gsb[:, s * 256:(s + 1) * 256], gp[:])
                    else:
                        nc.scalar.copy(gsb[:, s * 256:(s + 1) * 256], gp[:])
            nc.sync.dma_start(gin[:, :], gsb[:])
            nc.gpsimd.collective_compute(
                "AllReduce", ALU.add,
                replica_groups=[list(range(NC))],
                ins=[gin.opt()], outs=[gout.opt()],
            )
            nc.sync.dma_start(gout_s[:], gout[:, :])

            # ---- filter: U^T = W99 * G1^T (SBUF-only, vector+gpsimd) ----
            engs = (nc.vector, nc.gpsimd)
            for s in range(4):
                sl = slice(s * 256, (s + 1) * 256)
                engs[s % 2].tensor_mul(utb[:, sl], gout_s[:, sl],
                                       cbB_s[:, OF_W99 + s * 256: OF_W99 + (s + 1) * 256])

            # ---- B1: Z_t[a, c] = sum_b U_t[b, a] Qrows_t[c, b] ----
            # Zp (odd rows) at zbuf[0:512], Zm (even rows) at zbuf[512:1024]
            for am in range(2):
                zps = []
                for t in range(2):
                    zp = ps.tile([128, K], F32, tag="pp", bufs=4, name="pp")
                    for bm in range(2):
                        s = 2 * t + bm
                        nc.tensor.matmul(zp[:],
                                         utb[:, s * 256 + am * 128: s * 256 + (am + 1) * 128],
                                         cbB_s[:, OF_QROWST + s * 256: OF_QROWST + (s + 1) * 256],
                                         start=(bm == 0), stop=(bm == 1))
                    zps.append(zp)
                zsl = slice(am * 256, (am + 1) * 256)
                nc.scalar.copy(ztmp[:, zsl], zps[0][:])
                nc.vector.tensor_add(zbuf[:, am * 256: (am + 1) * 256], ztmp[:, zsl], zps[1][:])
                nc.vector.tensor_sub(zbuf[:, 512 + am * 256: 512 + (am + 1) * 256], ztmp[:, zsl], zps[1][:])

            # ---- B2: out chunks via parity-packed QloT ----
            dr_engs = (nc.scalar, nc.vector)
            di = 0
            for ic in range(8):
                for par in range(2):
                    ops = ps.tile([128, K], F32, tag="pp", bufs=4, name="pp")
                    zoff = 0 if par == 1 else 512
                    for am in range(2):
                        nc.tensor.matmul(ops[:],
                                         qcT_s[:, am * 2048 + par * 1024 + ic * 128: am * 2048 + par * 1024 + (ic + 1) * 128],
                                         zbuf[:, zoff + am * 256: zoff + (am + 1) * 256],
                                         start=(am == 0), stop=(am == 1))
                    dst = outb[:, (2 * ic + par) * 256: (2 * ic + par + 1) * 256]
                    eng = dr_engs[di % 2]; di += 1
                    if eng is nc.scalar:
                        eng.copy(dst, ops[:])
                    else:
                        eng.tensor_copy(dst, ops[:])
                if ic == 3:
                    nc.scalar.dma_start(out_d[:, 0:2048], outb[:, 0:2048])
            nc.scalar.dma_start(out_d[:, 2048:4096], outb[:, 2048:4096])

    nc.compile()
    _NC_CACHE["nc"] = nc
    return nc


def _run(X, Y, trace=False):
    _install_ntff_hook()
    from concourse.bass_utils import run_bass_kernel_spmd

    hc = _host_constants()
    Xf = np.asarray(X, np.float32).astype(np.float16)
    Yf = np.asarray(Y, np.float32).astype(np.float16)
    sgn = np.where(np.arange(128) % 2 == 1, 1.0, -1.0).astype(np.float16)

    in_maps = []
    for c in range(NC):
        xp = Xf[:, P * c: P * (c + 1)]           # [2048, 256]
        yp = Yf[:, P * c: P * (c + 1)]
        xyb = np.zeros((128, 8193), np.float16)
        xyb[:, 0] = sgn
        for r in range(RC):
            h, rr = r // 8, r % 8
            xyb[:, 1 + h * 4096 + rr * 256: 1 + h * 4096 + (rr + 1) * 256] = xp[128 * r:128 * (r + 1), :]
            xyb[:, 1 + h * 4096 + 2048 + rr * 256: 1 + h * 4096 + 2048 + (rr + 1) * 256] = yp[128 * r:128 * (r + 1), :]
        m = {"xyb": xyb,
             "qb": hc["qb"],
             "cbQ": _core_cbQ(c, hc),
             "cbB": _core_cbB(c, hc),
             "qcT": hc["qcT"]}
        in_maps.append(m)

    nc = _build()
    r = run_bass_kernel_spmd(nc, in_maps, core_ids=list(range(NC)), trace=trace)
    panels = []
    for c in range(NC):
        o = r.results[c]["out"].reshape(128, 8, 2, 256)      # [p, ic, par, c]
        panels.append(o.transpose(1, 0, 2, 3).reshape(2048, 256))
    full = np.concatenate(panels, axis=1).astype(np.float32)
    return full[None, None], r


def kernel(X, Y):
    out, _ = _run(X, Y, trace=False)
    return out
